# revision 5
# baseline (speedup 1.0000x reference)
"""Trainium2 Bass kernel for nn_BL_36721970381090 (dense_mlp).

Math (Kronecker-factored 2-layer MLP per batch row):
    z[b,d,u] = sum_s fc2_w[u,s] x[b,d,s]           (stage A, 200 outputs)
    h[b,t,u] = sum_d W11[t,d]  z[b,d,u]            (stage B, 600 outputs)
    y[b,o]   = sum_{t,u} W12[o,t] fc4[0,u] relu(h + bias1)[t,u] + bias2[o,0]

Mapping (data parallel over 8 cores, 16384 rows each; feature-major layout,
batch is the matmul moving/free dim, 512-column waves, 2048-column blocks):
  A: x chunks [100,512] x4 -> col-tiled concurrent pairs (tile_position
     (0,0)/(0,64)) into 2 PSUM banks; z rows (chunk: u,dl) d-major.
  z evac: scalar+vector PSUM->SBUF bf16 into a [128, 2048] block tile.
  permute: 20 SBUF->SBUF DMAs [10, 2048] per block -> u-major tiles
     (u even @rows 0-39, u odd @rows 64-103) for row-tiled stage B.
  B: 5 matmuls K=40 (shared W11^T stationary), row-tiled concurrent pairs
     (tile_position (0,0)/(64,0)), h_u [120,512] PSUM (5 banks).
  relu evac: scalar activation (bias+relu) x3 + vector add/max x2 -> bf16.
  C: 5 accumulating matmuls K=120 M=3 -> y [3,512] PSUM.
  y evac: vector copy -> SBUF, DMA out; host adds bias2.
"""

import numpy as np
import ml_dtypes
from contextlib import ExitStack

import concourse.bass as bass
import concourse.bacc as bacc
import concourse.mybir as mybir
from concourse.bass import ds
from concourse.tile import TileContext
from concourse.bass_utils import run_bass_kernel_spmd

B, D1, D2 = 131072, 40, 10
T0, T1, O0 = 120, 5, 3
NCORES = 8
BC = B // NCORES          # 16384 batch per core
KF = D1 * D2              # 400 input features
NB = 512                  # matmul free-dim wave
NBD = 2048                # block (4 waves)

F32 = mybir.dt.float32
BF16 = mybir.dt.bfloat16
BF = ml_dtypes.bfloat16
RELU = mybir.ActivationFunctionType.Relu
COPY = mybir.ActivationFunctionType.Copy
ADD = mybir.AluOpType.add
MAX = mybir.AluOpType.max

_CACHE = {}


def _build_nc():
    nc = bacc.Bacc()
    xt = nc.dram_tensor("xt", (KF, BC), BF16, kind="ExternalInput")
    ablk = nc.dram_tensor("ablk", (100, 64), BF16, kind="ExternalInput")
    w11t = nc.dram_tensor("w11t", (128, 120), BF16, kind="ExternalInput")
    cu = nc.dram_tensor("cu", (120, 16), BF16, kind="ExternalInput")
    b1u = nc.dram_tensor("b1u", (120, 8), F32, kind="ExternalInput")
    outT = nc.dram_tensor("outT", (O0, BC), F32, kind="ExternalOutput")

    with TileContext(nc) as tc, ExitStack() as ctx:
        consts = ctx.enter_context(tc.tile_pool(name="consts", bufs=1))
        a_sb = consts.tile([100, 64], BF16, tag="a")
        nc.sync.dma_start(a_sb[:, :], ablk[:, :])
        w_sb = consts.tile([128, 120], BF16, tag="w")
        nc.sync.dma_start(w_sb[:, :], w11t[:, :])
        c_sb = consts.tile([120, 16], BF16, tag="c")
        nc.sync.dma_start(c_sb[:, :], cu[:, :])
        b_sb = consts.tile([120, 8], F32, tag="b")
        nc.sync.dma_start(b_sb[:, :], b1u[:, :])

        xpool = ctx.enter_context(tc.tile_pool(name="xp", bufs=3))
        zdpool = ctx.enter_context(tc.tile_pool(name="zd", bufs=2))
        zupool = ctx.enter_context(tc.tile_pool(name="zu", bufs=2))
        rpool = ctx.enter_context(tc.tile_pool(name="rp", bufs=2))
        ypool = ctx.enter_context(tc.tile_pool(name="yp", bufs=2))
        psum = ctx.enter_context(tc.tile_pool(name="ps", bufs=1, space="PSUM"))
        za = psum.tile([128, NB], F32, tag="za")
        zb = psum.tile([128, NB], F32, tag="zb")
        hp = [psum.tile([120, NB], F32, tag=f"h{u}", name=f"hp{u}") for u in range(5)]
        yp = psum.tile([32, NB], F32, tag="y")

        for blk in range(BC // NBD):
            xk = [xpool.tile([100, NBD], BF16, tag=f"x{k}", name=f"xk{k}_{blk}")
                  for k in range(4)]
            for k in range(4):
                nc.sync.dma_start(xk[k][:, :], xt[ds(k * 100, 100), ds(blk * NBD, NBD)])
            zda = zdpool.tile([128, NBD], BF16, tag="zda", name=f"zda{blk}")
            zdb = zdpool.tile([128, NBD], BF16, tag="zdb", name=f"zdb{blk}")
            # phase 1: stage A + z evac per wave
            for jj in range(NBD // NB):
                cs = ds(jj * NB, NB)
                nc.tensor.matmul(za[0:50, :], a_sb[:, 0:50], xk[0][:, cs],
                                 start=True, stop=True, tile_position=(0, 0))
                nc.tensor.matmul(za[64:114, :], a_sb[:, 0:50], xk[1][:, cs],
                                 start=True, stop=True, tile_position=(0, 64))
                nc.tensor.matmul(zb[0:50, :], a_sb[:, 0:50], xk[2][:, cs],
                                 start=True, stop=True, tile_position=(0, 0))
                nc.tensor.matmul(zb[64:114, :], a_sb[:, 0:50], xk[3][:, cs],
                                 start=True, stop=True, tile_position=(0, 64))
                nc.scalar.activation(zda[0:114, cs], za[0:114, :], COPY)
                nc.vector.tensor_copy(zdb[0:114, cs], zb[0:114, :])
            # phase 2: permute to u-major (block granularity)
            zut = [zupool.tile([128, NBD], BF16, tag=f"zu{t}", name=f"zu{t}_{blk}")
                   for t in range(3)]
            for u in range(5):
                t, half = divmod(u, 2)
                for k in range(4):
                    s, kk = divmod(k, 2)
                    zd = (zda, zdb)[s]
                    nc.sync.dma_start(
                        zut[t][ds(half * 64 + k * 10, 10), :],
                        zd[ds(kk * 64 + u * 10, 10), :])
            # phase 3: B + relu + C + y evac per wave
            ysb = ypool.tile([O0, NBD], F32, tag="ysb", name=f"ysb{blk}")
            for jj in range(NBD // NB):
                cs = ds(jj * NB, NB)
                for u in range(5):
                    t, half = divmod(u, 2)
                    off = half * 64
                    nc.tensor.matmul(hp[u][0:120, :], w_sb[ds(off, 40), :],
                                     zut[t][ds(off, 40), cs],
                                     start=True, stop=True,
                                     tile_position=(off, 0))
                rt = [rpool.tile([120, NB], BF16, tag=f"r{u}", name=f"r{u}_{blk}_{jj}")
                      for u in range(5)]
                for u in range(3):
                    nc.scalar.activation(rt[u][:, :], hp[u][:, :], RELU,
                                         bias=b_sb[:, ds(u, 1)])
                for u in range(3, 5):
                    nc.vector.tensor_scalar(rt[u][:, :], hp[u][:, :],
                                            b_sb[:, ds(u, 1)], 0.0,
                                            op0=ADD, op1=MAX)
                for u in range(5):
                    nc.tensor.matmul(yp[0:3, :], c_sb[:, ds(3 * u, 3)],
                                     rt[u][:, :], start=(u == 0), stop=(u == 4),
                                     tile_position=(0, 0))
                nc.vector.tensor_copy(ysb[:, cs], yp[0:3, :])
            nc.sync.dma_start(outT[:, ds(blk * NBD, NBD)], ysb[:, :])
    nc.finalize()
    return nc


def kernel(x, W11, fc2_w, bias1, W12, fc4_w, bias2, _trace=False):
    x = np.asarray(x, dtype=np.float32)
    W11 = np.asarray(W11, np.float32)
    fc2_w = np.asarray(fc2_w, np.float32)
    bias1 = np.asarray(bias1, np.float32)
    W12 = np.asarray(W12, np.float32)
    fc4_w = np.asarray(fc4_w, np.float32)
    b2v = np.asarray(bias2, np.float32)[:, 0]

    # stage-A stationary: rows (dl, s), cols (u, dl') = fc2[u,s] * delta(dl,dl')
    A = np.zeros((100, 64), np.float32)
    A[:, :50] = np.einsum("us,de->dsue", fc2_w,
                          np.eye(10, dtype=np.float32)).reshape(100, 50)
    ablk = np.ascontiguousarray(A).astype(BF)
    # stage-B stationary: W11^T at rows 0-39 and 64-103
    w11t = np.zeros((128, 120), np.float32)
    w11t[0:40] = W11.T
    w11t[64:104] = W11.T
    w11t = np.ascontiguousarray(w11t).astype(BF)
    # stage-C stationaries: col group u = W12^T * fc4[0,u]
    cuv = np.zeros((120, 16), np.float32)
    for u in range(5):
        cuv[:, 3 * u:3 * u + 3] = W12.T * fc4_w[0, u]
    cuv = np.ascontiguousarray(cuv).astype(BF)
    # bias1 per-u columns
    b1u = np.zeros((120, 8), np.float32)
    b1u[:, 0:5] = bias1
    b1u = np.ascontiguousarray(b1u)

    if "nc" not in _CACHE:
        _CACHE["nc"] = _build_nc()
    nc = _CACHE["nc"]

    in_maps = []
    for c in range(NCORES):
        xs = x[c * BC:(c + 1) * BC]
        xtc = xs.transpose(1, 2, 0).reshape(KF, BC).astype(BF)
        in_maps.append({"xt": xtc, "ablk": ablk, "w11t": w11t,
                        "cu": cuv, "b1u": b1u})

    res = run_bass_kernel_spmd(nc, in_maps, core_ids=list(range(NCORES)),
                               trace=_trace)
    outs = [np.asarray(res.results[c]["outT"], np.float32) for c in range(NCORES)]
    full = np.concatenate(outs, axis=1)          # [3, B]
    y = full.T + b2v[None, :]
    if _trace:
        kernel.last_exec_time_ns = res.exec_time_ns
    return y.astype(np.float32)


# revision 6
# speedup vs baseline: 1.2231x; 1.2231x over previous
"""Trainium2 Bass kernel for nn_BL_36721970381090 (dense_mlp).

Math (Kronecker-factored 2-layer MLP per batch row):
    z[b,d,u] = sum_s fc2_w[u,s] x[b,d,s]           (stage A, 200 outputs)
    h[b,t,u] = sum_d W11[t,d]  z[b,d,u]            (stage B, 600 outputs)
    y[b,o]   = sum_{t,u} W12[o,t] fc4[0,u] relu(h + bias1)[t,u] + bias2[o,0]

Mapping (data parallel over 8 cores, 16384 rows each; feature-major layout,
batch is the matmul moving/free dim, 512-column waves, 2048-column blocks):
  A: x chunks [100,512] x4 -> col-tiled concurrent pairs (tile_position
     (0,0)/(0,64)) into 2 PSUM banks; z rows (chunk: u,dl) d-major.
  z evac: scalar+vector PSUM->SBUF bf16 into a [128, 2048] block tile.
  permute: 20 SBUF->SBUF DMAs [10, 2048] per block -> u-major tiles
     (u even @rows 0-39, u odd @rows 64-103) for row-tiled stage B.
  B: 5 matmuls K=40 (shared W11^T stationary), row-tiled concurrent pairs
     (tile_position (0,0)/(64,0)), h_u [120,512] PSUM (5 banks).
  relu evac: scalar activation (bias+relu) x3 + vector add/max x2 -> bf16.
  C: 5 accumulating matmuls K=120 M=3 -> y [3,512] PSUM.
  y evac: vector copy -> SBUF, DMA out; host adds bias2.
"""

import numpy as np
import ml_dtypes
from contextlib import ExitStack

import concourse.bass as bass
import concourse.bacc as bacc
import concourse.mybir as mybir
from concourse.bass import ds
from concourse.tile import TileContext
from concourse.bass_utils import run_bass_kernel_spmd

B, D1, D2 = 131072, 40, 10
T0, T1, O0 = 120, 5, 3
NCORES = 8
BC = B // NCORES          # 16384 batch per core
KF = D1 * D2              # 400 input features
NB = 512                  # matmul free-dim wave
NBD = 2048                # block (4 waves)

F32 = mybir.dt.float32
BF16 = mybir.dt.bfloat16
BF = ml_dtypes.bfloat16
RELU = mybir.ActivationFunctionType.Relu
COPY = mybir.ActivationFunctionType.Copy
ADD = mybir.AluOpType.add
MAX = mybir.AluOpType.max

_CACHE = {}


def _build_nc():
    nc = bacc.Bacc()
    xt = nc.dram_tensor("xt", (KF, BC), BF16, kind="ExternalInput")
    ablk = nc.dram_tensor("ablk", (100, 64), BF16, kind="ExternalInput")
    w11t = nc.dram_tensor("w11t", (128, 120), BF16, kind="ExternalInput")
    cu = nc.dram_tensor("cu", (120, 16), BF16, kind="ExternalInput")
    b1u = nc.dram_tensor("b1u", (120, 8), F32, kind="ExternalInput")
    outT = nc.dram_tensor("outT", (O0, BC), F32, kind="ExternalOutput")

    with TileContext(nc) as tc, ExitStack() as ctx:
        consts = ctx.enter_context(tc.tile_pool(name="consts", bufs=1))
        a_sb = consts.tile([100, 64], BF16, tag="a")
        nc.sync.dma_start(a_sb[:, :], ablk[:, :])
        w_sb = consts.tile([128, 120], BF16, tag="w")
        nc.sync.dma_start(w_sb[:, :], w11t[:, :])
        c_sb = consts.tile([120, 16], BF16, tag="c")
        nc.sync.dma_start(c_sb[:, :], cu[:, :])
        b_sb = consts.tile([120, 8], F32, tag="b")
        nc.sync.dma_start(b_sb[:, :], b1u[:, :])

        xpool = ctx.enter_context(tc.tile_pool(name="xp", bufs=3))
        zdpool = ctx.enter_context(tc.tile_pool(name="zd", bufs=2))
        zupool = ctx.enter_context(tc.tile_pool(name="zu", bufs=2))
        rpool = ctx.enter_context(tc.tile_pool(name="rp", bufs=2))
        ypool = ctx.enter_context(tc.tile_pool(name="yp", bufs=2))
        psum = ctx.enter_context(tc.tile_pool(name="ps", bufs=1, space="PSUM"))
        za = psum.tile([128, NB], F32, tag="za")
        zb = psum.tile([128, NB], F32, tag="zb")
        hp = [psum.tile([120, NB], F32, tag=f"h{u}", name=f"hp{u}") for u in range(5)]
        yp = psum.tile([32, NB], F32, tag="y")

        NBLK = BC // NBD
        NW = NBD // NB

        def load_x(blk):
            xk = [xpool.tile([100, NBD], BF16, tag=f"x{k}", name=f"xk{k}_{blk}")
                  for k in range(4)]
            for k in range(4):
                nc.sync.dma_start(xk[k][:, :],
                                  xt[ds(k * 100, 100), ds(blk * NBD, NBD)])
            return xk

        def stage_a_wave(xk, zda, zdb, jj):
            cs = ds(jj * NB, NB)
            nc.tensor.matmul(za[0:50, :], a_sb[:, 0:50], xk[0][:, cs],
                             start=True, stop=True, tile_position=(0, 0))
            nc.tensor.matmul(za[64:114, :], a_sb[:, 0:50], xk[1][:, cs],
                             start=True, stop=True, tile_position=(0, 64))
            nc.tensor.matmul(zb[0:50, :], a_sb[:, 0:50], xk[2][:, cs],
                             start=True, stop=True, tile_position=(0, 0))
            nc.tensor.matmul(zb[64:114, :], a_sb[:, 0:50], xk[3][:, cs],
                             start=True, stop=True, tile_position=(0, 64))
            nc.scalar.activation(zda[0:114, cs], za[0:114, :], COPY)
            nc.vector.tensor_copy(zdb[0:114, cs], zb[0:114, :])

        def permute(zda, zdb, blk):
            zut = [zupool.tile([128, NBD], BF16, tag=f"zu{t}", name=f"zu{t}_{blk}")
                   for t in range(3)]
            for u in range(5):
                t, half = divmod(u, 2)
                for k in range(4):
                    s, kk = divmod(k, 2)
                    zd = (zda, zdb)[s]
                    nc.sync.dma_start(
                        zut[t][ds(half * 64 + k * 10, 10), :],
                        zd[ds(kk * 64 + u * 10, 10), :])
            return zut

        def stage_bc_wave(zut, ysb, blk, jj):
            cs = ds(jj * NB, NB)
            for u in range(5):
                t, half = divmod(u, 2)
                off = half * 64
                nc.tensor.matmul(hp[u][0:120, :], w_sb[ds(off, 40), :],
                                 zut[t][ds(off, 40), cs],
                                 start=True, stop=True, tile_position=(off, 0))
            rt = [rpool.tile([120, NB], BF16, tag=f"r{u}", name=f"r{u}_{blk}_{jj}")
                  for u in range(5)]
            for u in range(3):
                nc.scalar.activation(rt[u][:, :], hp[u][:, :], RELU,
                                     bias=b_sb[:, ds(u, 1)])
            for u in range(3, 5):
                nc.vector.tensor_scalar(rt[u][:, :], hp[u][:, :],
                                        b_sb[:, ds(u, 1)], 0.0,
                                        op0=ADD, op1=MAX)
            for u in range(5):
                nc.tensor.matmul(yp[0:3, :], c_sb[:, ds(3 * u, 3)],
                                 rt[u][:, :], start=(u == 0), stop=(u == 4),
                                 tile_position=(0, 0))
            nc.vector.tensor_copy(ysb[:, cs], yp[0:3, :])

        def zd_tiles(blk):
            zda = zdpool.tile([128, NBD], BF16, tag="zda", name=f"zda{blk}")
            zdb = zdpool.tile([128, NBD], BF16, tag="zdb", name=f"zdb{blk}")
            return zda, zdb

        # software pipeline: A/z-evac of blk+1 interleaves with B/C of blk
        xk = load_x(0)
        xk_next = load_x(1)
        zda, zdb = zd_tiles(0)
        for jj in range(NW):
            stage_a_wave(xk, zda, zdb, jj)
        zut = permute(zda, zdb, 0)
        for blk in range(NBLK):
            ysb = ypool.tile([O0, NBD], F32, tag="ysb", name=f"ysb{blk}")
            if blk + 1 < NBLK:
                zda, zdb = zd_tiles(blk + 1)
                xk, xk_next = xk_next, None
            for jj in range(NW):
                stage_bc_wave(zut, ysb, blk, jj)
                if blk + 1 < NBLK:
                    stage_a_wave(xk, zda, zdb, jj)
            nc.sync.dma_start(outT[:, ds(blk * NBD, NBD)], ysb[:, :])
            if blk + 1 < NBLK:
                zut = permute(zda, zdb, blk + 1)
                if blk + 2 < NBLK:
                    xk_next = load_x(blk + 2)
    nc.finalize()
    return nc


def kernel(x, W11, fc2_w, bias1, W12, fc4_w, bias2, _trace=False):
    x = np.asarray(x, dtype=np.float32)
    W11 = np.asarray(W11, np.float32)
    fc2_w = np.asarray(fc2_w, np.float32)
    bias1 = np.asarray(bias1, np.float32)
    W12 = np.asarray(W12, np.float32)
    fc4_w = np.asarray(fc4_w, np.float32)
    b2v = np.asarray(bias2, np.float32)[:, 0]

    # stage-A stationary: rows (dl, s), cols (u, dl') = fc2[u,s] * delta(dl,dl')
    A = np.zeros((100, 64), np.float32)
    A[:, :50] = np.einsum("us,de->dsue", fc2_w,
                          np.eye(10, dtype=np.float32)).reshape(100, 50)
    ablk = np.ascontiguousarray(A).astype(BF)
    # stage-B stationary: W11^T at rows 0-39 and 64-103
    w11t = np.zeros((128, 120), np.float32)
    w11t[0:40] = W11.T
    w11t[64:104] = W11.T
    w11t = np.ascontiguousarray(w11t).astype(BF)
    # stage-C stationaries: col group u = W12^T * fc4[0,u]
    cuv = np.zeros((120, 16), np.float32)
    for u in range(5):
        cuv[:, 3 * u:3 * u + 3] = W12.T * fc4_w[0, u]
    cuv = np.ascontiguousarray(cuv).astype(BF)
    # bias1 per-u columns
    b1u = np.zeros((120, 8), np.float32)
    b1u[:, 0:5] = bias1
    b1u = np.ascontiguousarray(b1u)

    if "nc" not in _CACHE:
        _CACHE["nc"] = _build_nc()
    nc = _CACHE["nc"]

    in_maps = []
    for c in range(NCORES):
        xs = x[c * BC:(c + 1) * BC]
        xtc = xs.transpose(1, 2, 0).reshape(KF, BC).astype(BF)
        in_maps.append({"xt": xtc, "ablk": ablk, "w11t": w11t,
                        "cu": cuv, "b1u": b1u})

    res = run_bass_kernel_spmd(nc, in_maps, core_ids=list(range(NCORES)),
                               trace=_trace)
    outs = [np.asarray(res.results[c]["outT"], np.float32) for c in range(NCORES)]
    full = np.concatenate(outs, axis=1)          # [3, B]
    y = full.T + b2v[None, :]
    if _trace:
        kernel.last_exec_time_ns = res.exec_time_ns
    return y.astype(np.float32)
